# revision 11
# baseline (speedup 1.0000x reference)
"""CrossModalAttention TRN2 kernel: 8-core data-parallel (2 batches/core).

Hardcoded shapes B=16,S=512,H=768,NH=12,HD=64,MAX_REL=128 per the problem spec.

Design notes:
  - BitNet ternary linears: host quantizes W -> q in {-1,0,+1} (exact in f32r);
    the per-layer scale is a runtime input applied at PSUM evacuation.
  - Activations live transposed [feature, token]; every linear is
    out^T[o,n] = (W^T)[i,o].T @ x^T[i,n] with the ternary weight stationary.
  - f32r truncates BOTH matmul operands to 12 mantissa bits. Splitting an f32
    input into hi+lo 12-bit-exact halves makes a ternary-weight f32r matmul
    fp32-exact at 2 cyc/row (vs 4 for plain f32). Used for tp, q, k, v, tout —
    everything feeding the near-one-hot text softmax (logit sigma ~290) or the
    tight fused_text output. The attention core (scores, E^T transposes, av)
    runs in plain f32. i2tkv runs single-f32r (i2t softmax top-2 gap >= 41
    logits — robust to 1e-4 noise). Vision-path linears are tiny; plain f32.
  - Dead code in the reference: ln_v/v_n unused; t2i softmax is over a size-1
    axis (== 1.0) so t2i_q/ln_t2i are dead. text_mask is all-ones by
    construction in setup_inputs (fill "ones"); masking is a no-op. LN gains
    are ones / biases zeros by construction (asserted at runtime).
  - Relative position bias via a partition-shifted Toeplitz table:
    eshift[h][p, j] = rel_embed[clip(p - j + 384, -128, 128) + 128, h]; the
    bias tile for q-tile t is the pure AP slice eshift[h][:, 384-128t :][: 512].
Outputs are written transposed; the host transposes back and gathers cores.
"""
import sys
import types
from contextlib import ExitStack

import numpy as np

import concourse.bass as bass
import concourse.bass_utils as bass_utils
import concourse.mybir as mybir
import concourse.tile as tile
from concourse import bacc
from concourse.bass_utils import run_bass_kernel_spmd
from concourse.masks import make_identity

# This container's antenv lacks axon_hooks; shim it so trace=True works.
try:
    import antenv.axon_hooks  # noqa: F401
except ImportError:
    try:
        from trn_agent_boot.trn_boot import _ntff_profile_via_ctypes
        _m = types.ModuleType('antenv.axon_hooks')
        _hook = _ntff_profile_via_ctypes('/opt/axon/libaxon_pjrt.so')
        _m.get_axon_ntff_profile_hook = lambda: _hook
        sys.modules['antenv.axon_hooks'] = _m
        bass_utils.upload_artifacts = lambda d: d
    except Exception:
        pass

F32 = mybir.dt.float32
F32R = mybir.dt.float32r
F16 = mybir.dt.float16
AX = mybir.AxisListType
ACTF = mybir.ActivationFunctionType
OP = mybir.AluOpType

P = 128
B, S, H, NH = 16, 512, 768, 12
HD = H // NH           # 64
KT = H // P            # 6 feature tiles
QT = S // P            # 4 token tiles
BL = 2                 # batches per core
NCORES = 8
MAX_REL = 128
EPS = 1e-5
ESH_W = 896
ATT_SCALE = float(HD) ** -0.5

_CACHE = {}


def _quant(W):
    """BitNet eval quantization -> (ternary [in,out] f32, scale)."""
    W = np.asarray(W, np.float32)
    scale = np.float32(np.clip(np.float32(np.abs(W).mean()), 1e-5, 1000.0))
    wn = np.clip(W / scale, -10.0, 10.0)
    q = np.where(wn > np.float32(2.0 / 3.0), 1.0, 0.0) - \
        np.where(wn < -np.float32(2.0 / 3.0), 1.0, 0.0)
    return np.ascontiguousarray(q.T.astype(np.float32)), scale


def _col_tiles(x768):
    """[768] vector -> [128, 6] partition-major column tiles."""
    return np.ascontiguousarray(np.asarray(x768, np.float32).reshape(KT, P).T)


def _emit(nc, tc, t):
    ctx = ExitStack()
    cb = ctx.enter_context(tc.tile_pool(name="const", bufs=1))
    wp = ctx.enter_context(tc.tile_pool(name="w", bufs=4))
    wpa = ctx.enter_context(tc.tile_pool(name="wa", bufs=8))
    ep = ctx.enter_context(tc.tile_pool(name="eph", bufs=2))
    enp = ctx.enter_context(tc.tile_pool(name="enp", bufs=1))
    vp_ = ctx.enter_context(tc.tile_pool(name="vis", bufs=1))
    sm = ctx.enter_context(tc.tile_pool(name="sm", bufs=2))
    big = ctx.enter_context(tc.tile_pool(name="big", bufs=1))
    psA = ctx.enter_context(tc.tile_pool(name="psA", bufs=3, space="PSUM"))
    psT = ctx.enter_context(tc.tile_pool(name="psT", bufs=2, space="PSUM"))
    psV = ctx.enter_context(tc.tile_pool(name="psV", bufs=1, space="PSUM"))
    psL = ctx.enter_context(tc.tile_pool(name="psL", bufs=1, space="PSUM"))

    ident_f = cb.tile([P, P], F32, tag="ident_f")
    make_identity(nc, ident_f)
    ones_col = cb.tile([P, 1], F32, tag="ones_col")
    nc.vector.memset(ones_col, 1.0)
    eps_col = cb.tile([P, 1], F32, tag="eps_col")
    nc.vector.memset(eps_col, EPS)
    ones_row = cb.tile([1, P], F32, tag="ones_row")
    nc.vector.memset(ones_row, 1.0)

    # Toeplitz rel-bias table (f16: values ~N(0,1), abs err ~2e-4 logits)
    esh = cb.tile([P, NH, ESH_W], F16, tag="esh")
    nc.sync.dma_start(out=esh, in_=t["eshift"].ap().rearrange("h p w -> p h w"))

    bias = {}
    for nm in ("b_tp", "b_q", "b_k", "b_tout", "b_vp", "b_i2tq",
               "b_t2ikv_v", "b_vout"):
        bt = cb.tile([P, KT], F32, tag=nm)
        nc.sync.dma_start(out=bt, in_=t[nm].ap())
        bias[nm] = bt
    b_i2tkv = cb.tile([P, KT], F32, tag="b_i2tkv")
    nc.sync.dma_start(out=b_i2tkv, in_=t["b_i2tkv"].ap())
    bias["b_i2tkv"] = b_i2tkv

    NSC = 12
    scbc = cb.tile([P, NSC], F32, tag="scbc")
    srcap = t["scal"].ap()
    nc.sync.dma_start(
        out=scbc,
        in_=bass.AP(tensor=srcap.tensor, offset=srcap.offset,
                    ap=[[0, P], [1, NSC]]))
    (SC_TP, SC_QKV, SC_I2TKV, SC_TOUT, SC_VP, SC_I2TQ, SC_T2IKV, SC_VOUT,
     AL_I2T, AL_T2I) = range(10)

    def sc(i):
        return scbc[:, i:i + 1]

    def wtile(dram, kk, mm, dt, tag, mw=P, coff=0):
        pool = wpa if tag == "wA" else wp
        wt = pool.tile([P, mw], dt, tag=tag)
        nc.sync.dma_start(
            out=wt,
            in_=dram.ap()[kk * P:(kk + 1) * P, coff + mm * mw:coff + (mm + 1) * mw])
        return wt

    def lin_splitA(dram, hi, lo, out_sb, scale_ap, bias_t, n):
        """out^T[:,m,:] = scale*(W^T.T @ (hi+lo)) + bias, ternary W in f32r."""
        for m in range(KT):
            pt = psA.tile([P, S], F32, tag="A")
            for k in range(KT):
                wt = wtile(dram, k, m, F32R, "wA")
                nc.tensor.matmul(pt[:, :n], lhsT=wt, rhs=hi[:, k, :],
                                 start=(k == 0), stop=False)
                nc.tensor.matmul(pt[:, :n], lhsT=wt, rhs=lo[:, k, :],
                                 start=False, stop=(k == KT - 1))
            nc.scalar.activation(out=out_sb[:, m, :], in_=pt[:, :n],
                                 func=ACTF.Identity, bias=bias_t[:, m:m + 1],
                                 scale=scale_ap)

    def split_hi_lo(src_sb, hi, lo):
        for m in range(KT):
            nc.gpsimd.tensor_copy(hi[:, m, :], src_sb[:, m, :])
            nc.vector.tensor_tensor(out=lo[:, m, :], in0=src_sb[:, m, :],
                                    in1=hi[:, m, :].bitcast(F32), op=OP.subtract)

    def ln_T(src, dst, n):
        """LayerNorm over the feature (partition) dim of src [128,KT,n]."""
        stat0 = psL.tile([1, S], F32, tag="L0")
        stat1 = psL.tile([1, S], F32, tag="L1")
        sq = ep.tile([P, S], F32, tag="ln_sq")
        for k in range(KT):
            nc.tensor.matmul(stat0[:, :n], lhsT=ones_col, rhs=src[:, k, :],
                             start=(k == 0), stop=(k == KT - 1))
        for k in range(KT):
            nc.scalar.activation(out=sq[:, :n], in_=src[:, k, :], func=ACTF.Square)
            nc.tensor.matmul(stat1[:, :n], lhsT=ones_col, rhs=sq[:, :n],
                             start=(k == 0), stop=(k == KT - 1))
        row0 = sm.tile([1, S], F32, tag="ln_row0")
        row1 = sm.tile([1, S], F32, tag="ln_row1")
        nc.vector.tensor_scalar(out=row0[:, :n], in0=stat0[:, :n],
                                scalar1=1.0 / H, scalar2=None, op0=OP.mult)
        nc.vector.tensor_scalar(out=row1[:, :n], in0=stat1[:, :n],
                                scalar1=1.0 / H, scalar2=None, op0=OP.mult)
        var = sm.tile([1, S], F32, tag="ln_var")
        nc.vector.tensor_tensor(out=var[:, :n], in0=row0[:, :n],
                                in1=row0[:, :n], op=OP.mult)
        nc.vector.tensor_tensor(out=var[:, :n], in0=row1[:, :n],
                                in1=var[:, :n], op=OP.subtract)
        sg = sm.tile([1, S], F32, tag="ln_sg")
        nc.scalar.activation(out=sg[:, :n], in_=var[:, :n], func=ACTF.Sqrt,
                             bias=eps_col[0:1, :])
        rs = sm.tile([1, S], F32, tag="ln_rs")
        nc.vector.reciprocal(out=rs[:, :n], in_=sg[:, :n])
        tmp = sm.tile([1, S], F32, tag="ln_tmp")
        nc.vector.tensor_tensor(out=tmp[:, :n], in0=sg[:, :n], in1=rs[:, :n],
                                op=OP.mult)
        nc.vector.tensor_scalar(out=tmp[:, :n], in0=tmp[:, :n], scalar1=-1.0,
                                scalar2=2.0, op0=OP.mult, op1=OP.add)
        nc.vector.tensor_tensor(out=rs[:, :n], in0=rs[:, :n], in1=tmp[:, :n],
                                op=OP.mult)
        mub = psL.tile([P, S], F32, tag="L0")
        rsb = psL.tile([P, S], F32, tag="L1")
        nc.tensor.matmul(mub[:, :n], lhsT=ones_row, rhs=row0[:, :n],
                         start=True, stop=True)
        nc.tensor.matmul(rsb[:, :n], lhsT=ones_row, rhs=rs[0:1, :n],
                         start=True, stop=True)
        for k in range(KT):
            nc.vector.tensor_tensor(out=dst[:, k, :], in0=src[:, k, :],
                                    in1=mub[:, :n], op=OP.subtract)
            nc.vector.tensor_tensor(out=dst[:, k, :], in0=dst[:, k, :],
                                    in1=rsb[:, :n], op=OP.mult)

    # ---------------- vision path (both batches, tiny) ----------------
    vis_nat = vp_.tile([BL, H], F32, tag="vis_nat")
    nc.sync.dma_start(out=vis_nat, in_=t["vision"].ap())
    visT = vp_.tile([P, KT, BL], F32, tag="visT")
    for k in range(KT):
        pt = psV.tile([P, NH], F32, tag="V")
        nc.tensor.transpose(pt[:, :BL], vis_nat[:, k * P:(k + 1) * P], ident_f[:BL, :BL])
        nc.vector.tensor_copy(visT[:, k, :], pt[:, :BL])

    def vis_lin(wname, bname, src, scale_i):
        dst = vp_.tile([P, KT, BL], F32, tag=f"vl_{wname}")
        for m in range(KT):
            pt = psV.tile([P, NH], F32, tag="V")
            for k in range(KT):
                wt = wtile(t[wname], k, m, F32, "wV")
                nc.tensor.matmul(pt[:, :BL], lhsT=wt, rhs=src[:, k, :],
                                 start=(k == 0), stop=(k == KT - 1))
            nc.scalar.activation(out=dst[:, m, :], in_=pt[:, :BL],
                                 func=ACTF.Identity, bias=bias[bname][:, m:m + 1],
                                 scale=sc(scale_i))
        return dst

    vpT = vis_lin("w_vp", "b_vp", visT, SC_VP)          # v_exp^T
    vlnT = vp_.tile([P, KT, BL], F32, tag="vlnT")
    ln_T(vpT, vlnT, BL)
    i2tqT = vis_lin("w_i2tq", "b_i2tq", vlnT, SC_I2TQ)
    t2ivT = vis_lin("w_t2ikv_v", "b_t2ikv_v", vpT, SC_T2IKV)
    a2T = vp_.tile([P, KT, BL], F32, tag="a2T")
    for k in range(KT):
        nc.vector.tensor_scalar(out=a2T[:, k, :], in0=t2ivT[:, k, :],
                                scalar1=sc(AL_T2I), scalar2=None, op0=OP.mult)
    vcrossT = vp_.tile([P, KT, BL], F32, tag="vcrossT")

    textr = t["text"].ap().rearrange("b (qt p) h -> b qt p h", p=P)

    # ---------------- text path (per batch) ----------------
    for b in range(BL):
        textT = big.tile([P, KT, S], F32, tag="bigA")
        for qt in range(QT):
            nat = ep.tile([P, H], F32, tag="t_nat")
            nc.sync.dma_start(out=nat, in_=textr[b, qt])
            for k in range(KT):
                pt = psT.tile([P, S], F32, tag="T")
                nc.tensor.transpose(pt[:, :P], nat[:, k * P:(k + 1) * P], ident_f)
                nc.vector.tensor_copy(textT[:, k, qt * P:(qt + 1) * P],
                                      pt[:, :P])
        x_hi = big.tile([P, KT, S], F32R, tag="bigB")
        x_lo = big.tile([P, KT, S], F32R, tag="bigC")
        split_hi_lo(textT, x_hi, x_lo)
        tpT = big.tile([P, KT, S], F32, tag="tpT")
        lin_splitA(t["w_tp"], x_hi, x_lo, tpT, sc(SC_TP), bias["b_tp"], S)
        tnT = big.tile([P, KT, S], F32, tag="bigA")
        ln_T(tpT, tnT, S)
        tn_hi = big.tile([P, KT, S], F32R, tag="bigB")
        tn_lo = big.tile([P, KT, S], F32R, tag="bigC")
        split_hi_lo(tnT, tn_hi, tn_lo)
        qT = big.tile([P, KT, S], F32, tag="qT")
        kTT = big.tile([P, KT, S], F32, tag="kT")
        lin_splitA(t["w_q"], tn_hi, tn_lo, qT, sc(SC_QKV), bias["b_q"], S)
        lin_splitA(t["w_k"], tn_hi, tn_lo, kTT, sc(SC_QKV), bias["b_k"], S)
        # v natural via FORM B (v bias is all-zero; asserted host-side)
        vN = big.tile([P, QT, H], F32, tag="vN")
        for qt in range(QT):
            for half in range(2):
                pt = psA.tile([P, S], F32, tag="A")
                for k in range(KT):
                    wt = wtile(t["w_v"], k, half, F32R, "wB", mw=384)
                    nc.tensor.matmul(pt[:, :384],
                                     lhsT=tn_hi[:, k, qt * P:(qt + 1) * P],
                                     rhs=wt, start=(k == 0), stop=False)
                    nc.tensor.matmul(pt[:, :384],
                                     lhsT=tn_lo[:, k, qt * P:(qt + 1) * P],
                                     rhs=wt, start=False, stop=(k == KT - 1))
                nc.scalar.activation(
                    out=vN[:, qt, half * 384:(half + 1) * 384],
                    in_=pt[:, :384], func=ACTF.Identity, scale=sc(SC_QKV))

        # ------------- self-attention -------------
        tsT = big.tile([P, KT, S], F32, tag="tsT")
        for h in range(NH):
            kt_i, pof = h // 2, (h % 2) * HD
            qh = qT[pof:pof + HD, kt_i, :]
            kh = kTT[pof:pof + HD, kt_i, :]
            EnT = enp.tile([P, QT, S], F32, tag="EnT")
            for qt in range(QT):
                spt = psA.tile([P, S], F32, tag="A")
                nc.tensor.matmul(spt, lhsT=qh[:, qt * P:(qt + 1) * P], rhs=kh,
                                 start=True, stop=True)
                sb_s = ep.tile([P, S], F32, tag="sb_s")
                nc.vector.tensor_scalar(out=sb_s, in0=spt, scalar1=ATT_SCALE,
                                        scalar2=None, op0=OP.mult)
                nc.vector.tensor_tensor(
                    out=sb_s, in0=sb_s,
                    in1=esh[:, h, 384 - P * qt: 384 - P * qt + S], op=OP.add)
                mx = sm.tile([P, 1], F32, tag="mx")
                nc.vector.reduce_max(mx, sb_s, axis=AX.X)
                nmx = sm.tile([P, 1], F32, tag="nmx")
                nc.gpsimd.tensor_scalar(out=nmx, in0=mx, scalar1=-1.0,
                                         scalar2=None, op0=OP.mult)
                Et = ep.tile([P, S], F32, tag="Et")
                sums = sm.tile([P, 1], F32, tag="sums")
                nc.scalar.activation(out=Et, in_=sb_s, func=ACTF.Exp, bias=nmx,
                                     scale=1.0, accum_out=sums)
                rec = sm.tile([P, 1], F32, tag="rec")
                nc.vector.reciprocal(out=rec, in_=sums)
                t1 = sm.tile([P, 1], F32, tag="t1")
                nc.vector.tensor_tensor(out=t1, in0=sums, in1=rec, op=OP.mult)
                nc.vector.tensor_scalar(out=t1, in0=t1, scalar1=-1.0,
                                        scalar2=2.0, op0=OP.mult, op1=OP.add)
                nc.vector.tensor_tensor(out=rec, in0=rec, in1=t1, op=OP.mult)
                En = ep.tile([P, S], F32, tag="En")
                nc.gpsimd.tensor_scalar(out=En, in0=Et, scalar1=rec,
                                        scalar2=None, op0=OP.mult)
                for c in range(QT):
                    ptt = psT.tile([P, S], F32, tag="T")
                    nc.tensor.transpose(ptt[:, :P], En[:, c * P:(c + 1) * P],
                                        ident_f)
                    nc.vector.tensor_copy(EnT[:, c, qt * P:(qt + 1) * P],
                                          ptt[:, :P])
            apt = psT.tile([P, S], F32, tag="T")
            for c in range(QT):
                nc.tensor.matmul(apt[:HD, :], lhsT=vN[:, c, h * HD:(h + 1) * HD],
                                 rhs=EnT[:, c, :], start=(c == 0),
                                 stop=(c == QT - 1))
            nc.vector.tensor_tensor(out=tsT[pof:pof + HD, kt_i, :],
                                    in0=apt[:HD, :],
                                    in1=tpT[pof:pof + HD, kt_i, :], op=OP.add)

        # ------------- i2t cross attention + outputs -------------
        tsr = big.tile([P, KT, S], F32R, tag="bigA")
        for k in range(KT):
            nc.gpsimd.tensor_copy(tsr[:, k, :], tsT[:, k, :])
        i2tkT = big.tile([P, KT, S], F32, tag="bigB")
        for m in range(KT):
            pt = psA.tile([P, S], F32, tag="A")
            for k in range(KT):
                wt = wtile(t["w_i2tkv"], k, m, F32R, "wA")
                nc.tensor.matmul(pt, lhsT=wt, rhs=tsr[:, k, :],
                                 start=(k == 0), stop=(k == KT - 1))
            nc.scalar.activation(out=i2tkT[:, m, :], in_=pt, func=ACTF.Identity,
                                 bias=bias["b_i2tkv"][:, m:m + 1],
                                 scale=sc(SC_I2TKV))
        i2tvN = big.tile([P, QT, H], F32, tag="bigC")
        for qt in range(QT):
            for half in range(2):
                pt = psA.tile([P, S], F32, tag="A")
                for k in range(KT):
                    wt = wtile(t["w_i2tkv"], k, half, F32R, "wB", mw=384,
                               coff=H)
                    nc.tensor.matmul(pt[:, :384],
                                     lhsT=tsr[:, k, qt * P:(qt + 1) * P],
                                     rhs=wt, start=(k == 0),
                                     stop=(k == KT - 1))
                nc.scalar.activation(
                    out=i2tvN[:, qt, half * 384:(half + 1) * 384],
                    in_=pt[:, :384], func=ACTF.Identity, scale=sc(SC_I2TKV))
        isb = sm.tile([NH, S], F32, tag="isb")
        for h in range(NH):
            kt_i, pof = h // 2, (h % 2) * HD
            ispt = psT.tile([1, S], F32, tag="T")
            nc.tensor.matmul(ispt,
                             lhsT=i2tqT[pof:pof + HD, kt_i, b:b + 1],
                             rhs=i2tkT[pof:pof + HD, kt_i, :],
                             start=True, stop=True)
            isr = sm.tile([1, S], F32, tag="isr")
            nc.vector.tensor_scalar(out=isr, in0=ispt, scalar1=ATT_SCALE,
                                    scalar2=None, op0=OP.mult)
            nc.sync.dma_start(out=isb[h:h + 1, :], in_=isr)
        imx = sm.tile([NH, 1], F32, tag="imx")
        nc.vector.reduce_max(imx, isb, axis=AX.X)
        nc.vector.tensor_scalar(out=imx, in0=imx, scalar1=-1.0, scalar2=None,
                                op0=OP.mult)
        iE = sm.tile([NH, S], F32, tag="iE")
        isums = sm.tile([NH, 1], F32, tag="isums")
        nc.scalar.activation(out=iE, in_=isb, func=ACTF.Exp, bias=imx,
                             scale=1.0, accum_out=isums)
        irec = sm.tile([NH, 1], F32, tag="irec")
        nc.vector.reciprocal(out=irec, in_=isums)
        nc.vector.tensor_scalar(out=iE, in0=iE, scalar1=irec, scalar2=None,
                                op0=OP.mult)
        iaT = sm.tile([P, QT, NH], F32, tag="iaT")
        for c in range(QT):
            ptt = psV.tile([P, NH], F32, tag="V")
            nc.tensor.transpose(ptt[:, :NH], iE[:, c * P:(c + 1) * P], ident_f[:NH, :NH])
            nc.vector.tensor_copy(iaT[:, c, :], ptt[:, :NH])
        for m in range(KT):
            pt = psV.tile([P, NH], F32, tag="V")
            for c in range(QT):
                nc.tensor.matmul(pt, lhsT=i2tvN[:, c, m * P:(m + 1) * P],
                                 rhs=iaT[:, c, :], start=(c == 0),
                                 stop=(c == QT - 1))
            for half in range(2):
                h = 2 * m + half
                nc.vector.tensor_scalar(
                    out=vcrossT[half * HD:(half + 1) * HD, m, b:b + 1],
                    in0=pt[half * HD:(half + 1) * HD, h:h + 1],
                    scalar1=scbc[half * HD:(half + 1) * HD, AL_I2T:AL_I2T + 1],
                    scalar2=None, op0=OP.mult)
                nc.vector.tensor_tensor(
                    out=vcrossT[half * HD:(half + 1) * HD, m, b:b + 1],
                    in0=vcrossT[half * HD:(half + 1) * HD, m, b:b + 1],
                    in1=vpT[half * HD:(half + 1) * HD, m, b:b + 1], op=OP.add)

        # text_cross = ts + alpha_t2i * t2i_v (broadcast over tokens), in place
        for k in range(KT):
            nc.vector.tensor_scalar(out=tsT[:, k, :], in0=tsT[:, k, :],
                                    scalar1=a2T[:, k, b:b + 1], scalar2=None,
                                    op0=OP.add)
        tc_hi = big.tile([P, KT, S], F32R, tag="bigA")
        tc_lo = big.tile([P, KT, S], F32R, tag="bigC")
        split_hi_lo(tsT, tc_hi, tc_lo)
        ftT = big.tile([P, KT, S], F32, tag="bigB")
        lin_splitA(t["w_tout"], tc_hi, tc_lo, ftT, sc(SC_TOUT),
                   bias["b_tout"], S)
        nc.sync.dma_start(
            out=t["out_ft"].ap().rearrange("b (kt p) s -> b p kt s", p=P)[b],
            in_=ftT)

    fvT = vis_lin("w_vout", "b_vout", vcrossT, SC_VOUT)
    nc.sync.dma_start(
        out=t["out_fv"].ap().rearrange("(kt p) b -> p kt b", p=P), in_=fvT)
    ctx.close()


def _build():
    nc = bacc.Bacc(None, target_bir_lowering=False)

    def din(name, shape, dt=F32):
        return nc.declare_dram_parameter(name, list(shape), dt, isOutput=False)

    t = {}
    t["text"] = din("text", [BL, S, H])
    t["vision"] = din("vision", [BL, H])
    t["w_tp"] = din("w_tp", [H, H], F32R)
    t["w_q"] = din("w_q", [H, H], F32R)
    t["w_k"] = din("w_k", [H, H], F32R)
    t["w_v"] = din("w_v", [H, H], F32R)
    t["w_i2tkv"] = din("w_i2tkv", [H, 2 * H], F32R)
    t["w_tout"] = din("w_tout", [H, H], F32R)
    t["w_vp"] = din("w_vp", [H, H])
    t["w_i2tq"] = din("w_i2tq", [H, H])
    t["w_t2ikv_v"] = din("w_t2ikv_v", [H, H])
    t["w_vout"] = din("w_vout", [H, H])
    t["eshift"] = din("eshift", [NH, P, ESH_W], F16)
    for nm in ("b_tp", "b_q", "b_k", "b_tout", "b_vp", "b_i2tq", "b_t2ikv_v",
               "b_vout", "b_i2tkv"):
        t[nm] = din(nm, [P, KT])
    t["scal"] = din("scal", [12, 1])
    t["out_ft"] = nc.declare_dram_parameter("out_ft", [BL, H, S], F32,
                                            isOutput=True)
    t["out_fv"] = nc.declare_dram_parameter("out_fv", [H, BL], F32,
                                            isOutput=True)
    with tile.TileContext(nc) as tc:
        _emit(nc, tc, t)
    nc.finalize()
    return nc


def kernel(**inputs):
    f32 = lambda x: np.ascontiguousarray(np.asarray(x, np.float32))
    for n in ("v", "t", "i2t", "t2i"):
        assert np.all(np.asarray(inputs[f"ln_{n}_g"]) == 1.0)
        assert np.all(np.asarray(inputs[f"ln_{n}_b"]) == 0.0)
    assert np.all(np.asarray(inputs["text_mask"]))
    assert np.all(np.asarray(inputs["tqkv_b"])[2 * H:] == 0)
    assert np.all(np.asarray(inputs["i2tkv_b"])[H:] == 0)

    nc = _CACHE.get("nc")
    if nc is None:
        nc = _build()
        _CACHE["nc"] = nc

    q_tp, s_tp = _quant(inputs["tp_w"])
    q_qkv, s_qkv = _quant(inputs["tqkv_w"])
    q_i2tkv, s_i2tkv = _quant(inputs["i2tkv_w"])
    q_tout, s_tout = _quant(inputs["tout_w"])
    q_vp, s_vp = _quant(inputs["vp_w"])
    q_i2tq, s_i2tq = _quant(inputs["i2tq_w"])
    q_t2ikv, s_t2ikv = _quant(inputs["t2ikv_w"])
    q_vout, s_vout = _quant(inputs["vout_w"])

    emb = f32(inputs["rel_embed"])
    jj = np.arange(ESH_W)[None, :]
    pp = np.arange(P)[:, None]
    idx = np.clip(pp - jj + 384, -MAX_REL, MAX_REL) + MAX_REL
    esh = np.ascontiguousarray(
        emb[idx][None].transpose(3, 1, 2, 0)[..., 0].astype(np.float16))
    # esh[h, p, j] = emb[idx[p, j], h]
    esh = np.ascontiguousarray(emb[idx, :].transpose(2, 0, 1).astype(np.float16))

    qkv_b = f32(inputs["tqkv_b"])
    scal = np.array([s_tp, s_qkv, s_i2tkv, s_tout, s_vp, s_i2tq, s_t2ikv,
                     s_vout, float(np.asarray(inputs["alpha_i2t"]).ravel()[0]),
                     float(np.asarray(inputs["alpha_t2i"]).ravel()[0]),
                     0.0, 0.0], np.float32).reshape(12, 1)

    base = {
        "w_tp": q_tp,
        "w_q": np.ascontiguousarray(q_qkv[:, :H]),
        "w_k": np.ascontiguousarray(q_qkv[:, H:2 * H]),
        "w_v": np.ascontiguousarray(q_qkv[:, 2 * H:]),
        "w_i2tkv": q_i2tkv, "w_tout": q_tout, "w_vp": q_vp,
        "w_i2tq": q_i2tq,
        "w_t2ikv_v": np.ascontiguousarray(q_t2ikv[:, H:]),
        "w_vout": q_vout, "eshift": esh, "scal": scal,
        "b_tp": _col_tiles(inputs["tp_b"]),
        "b_q": _col_tiles(qkv_b[:H]),
        "b_k": _col_tiles(qkv_b[H:2 * H]),
        "b_i2tkv": _col_tiles(f32(inputs["i2tkv_b"])[:H]),
        "b_tout": _col_tiles(inputs["tout_b"]),
        "b_vp": _col_tiles(inputs["vp_b"]),
        "b_i2tq": _col_tiles(inputs["i2tq_b"]),
        "b_t2ikv_v": _col_tiles(f32(inputs["t2ikv_b"])[H:]),
        "b_vout": _col_tiles(inputs["vout_b"]),
    }
    text = f32(inputs["text_features"])
    visf = f32(inputs["vision_features"])
    in_maps = []
    for c in range(NCORES):
        m = dict(base)
        m["text"] = np.ascontiguousarray(text[c * BL:(c + 1) * BL])
        m["vision"] = np.ascontiguousarray(visf[c * BL:(c + 1) * BL])
        in_maps.append(m)
    _CACHE["in_maps"] = in_maps

    res = run_bass_kernel_spmd(nc, in_maps, core_ids=list(range(NCORES)))
    fv = np.empty((B, H), np.float32)
    ft = np.empty((B, S, H), np.float32)
    for c in range(NCORES):
        r = res.results[c]
        fv[c * BL:(c + 1) * BL] = r["out_fv"].T
        ft[c * BL:(c + 1) * BL] = r["out_ft"].transpose(0, 2, 1)
    return fv, ft


# revision 12
# speedup vs baseline: 1.8023x; 1.8023x over previous
"""CrossModalAttention TRN2 kernel: 8-core data-parallel (2 batches/core).

Hardcoded shapes B=16,S=512,H=768,NH=12,HD=64,MAX_REL=128 per the problem spec.

Design notes:
  - BitNet ternary linears: host quantizes W -> q in {-1,0,+1} (exact in f32r);
    the per-layer scale is a runtime input applied at PSUM evacuation.
  - Activations live transposed [feature, token]; every linear is
    out^T[o,n] = (W^T)[i,o].T @ x^T[i,n] with the ternary weight stationary.
  - f32r truncates BOTH matmul operands to 12 mantissa bits. Splitting an f32
    input into hi+lo 12-bit-exact halves makes a ternary-weight f32r matmul
    fp32-exact at 2 cyc/row (vs 4 for plain f32). Used for tp, q, k, v, tout —
    everything feeding the near-one-hot text softmax (logit sigma ~290) or the
    tight fused_text output. The attention core (scores, E^T transposes, av)
    runs in plain f32. i2tkv runs single-f32r (i2t softmax top-2 gap >= 41
    logits — robust to 1e-4 noise). Vision-path linears are tiny; plain f32.
  - Dead code in the reference: ln_v/v_n unused; t2i softmax is over a size-1
    axis (== 1.0) so t2i_q/ln_t2i are dead. text_mask is all-ones by
    construction in setup_inputs (fill "ones"); masking is a no-op. LN gains
    are ones / biases zeros by construction (asserted at runtime).
  - Relative position bias via a partition-shifted Toeplitz table:
    eshift[h][p, j] = rel_embed[clip(p - j + 384, -128, 128) + 128, h]; the
    bias tile for q-tile t is the pure AP slice eshift[h][:, 384-128t :][: 512].
Outputs are written transposed; the host transposes back and gathers cores.
"""
import sys
import types
from contextlib import ExitStack

import numpy as np

import concourse.bass as bass
import concourse.bass_utils as bass_utils
import concourse.mybir as mybir
import concourse.tile as tile
from concourse import bacc
from concourse.bass_utils import run_bass_kernel_spmd
from concourse.masks import make_identity

# This container's antenv lacks axon_hooks; shim it so trace=True works.
try:
    import antenv.axon_hooks  # noqa: F401
except ImportError:
    try:
        from trn_agent_boot.trn_boot import _ntff_profile_via_ctypes
        _m = types.ModuleType('antenv.axon_hooks')
        _hook = _ntff_profile_via_ctypes('/opt/axon/libaxon_pjrt.so')
        _m.get_axon_ntff_profile_hook = lambda: _hook
        sys.modules['antenv.axon_hooks'] = _m
        bass_utils.upload_artifacts = lambda d: d
    except Exception:
        pass

F32 = mybir.dt.float32
F32R = mybir.dt.float32r
F16 = mybir.dt.float16
AX = mybir.AxisListType
ACTF = mybir.ActivationFunctionType
OP = mybir.AluOpType

P = 128
B, S, H, NH = 16, 512, 768, 12
HD = H // NH           # 64
KT = H // P            # 6 feature tiles
QT = S // P            # 4 token tiles
BL = 2                 # batches per core
NCORES = 8
MAX_REL = 128
EPS = 1e-5
ESH_W = 896
ATT_SCALE = float(HD) ** -0.5

_CACHE = {}


def _quant(W):
    """BitNet eval quantization -> (ternary [in,out] f32, scale)."""
    W = np.asarray(W, np.float32)
    scale = np.float32(np.clip(np.float32(np.abs(W).mean()), 1e-5, 1000.0))
    wn = np.clip(W / scale, -10.0, 10.0)
    q = np.where(wn > np.float32(2.0 / 3.0), 1.0, 0.0) - \
        np.where(wn < -np.float32(2.0 / 3.0), 1.0, 0.0)
    return np.ascontiguousarray(q.T.astype(np.float32)), scale


def _col_tiles(x768):
    """[768] vector -> [128, 6] partition-major column tiles."""
    return np.ascontiguousarray(np.asarray(x768, np.float32).reshape(KT, P).T)


def _emit(nc, tc, t):
    ctx = ExitStack()
    cb = ctx.enter_context(tc.tile_pool(name="const", bufs=1))
    wp = ctx.enter_context(tc.tile_pool(name="w", bufs=4))
    wpa = ctx.enter_context(tc.tile_pool(name="wa", bufs=8))
    ep = ctx.enter_context(tc.tile_pool(name="eph", bufs=2))
    enp = ctx.enter_context(tc.tile_pool(name="enp", bufs=1))
    vp_ = ctx.enter_context(tc.tile_pool(name="vis", bufs=1))
    sm = ctx.enter_context(tc.tile_pool(name="sm", bufs=2))
    big = ctx.enter_context(tc.tile_pool(name="big", bufs=1))
    psA = ctx.enter_context(tc.tile_pool(name="psA", bufs=3, space="PSUM"))
    psT = ctx.enter_context(tc.tile_pool(name="psT", bufs=2, space="PSUM"))
    psV = ctx.enter_context(tc.tile_pool(name="psV", bufs=1, space="PSUM"))
    psL = ctx.enter_context(tc.tile_pool(name="psL", bufs=1, space="PSUM"))

    ident_f = cb.tile([P, P], F32, tag="ident_f")
    make_identity(nc, ident_f)
    ones_col = cb.tile([P, 1], F32, tag="ones_col")
    nc.vector.memset(ones_col, 1.0)
    eps_col = cb.tile([P, 1], F32, tag="eps_col")
    nc.vector.memset(eps_col, EPS)
    ones_row = cb.tile([1, P], F32, tag="ones_row")
    nc.vector.memset(ones_row, 1.0)

    # Toeplitz rel-bias table (f16: values ~N(0,1), abs err ~2e-4 logits)
    esh = cb.tile([P, NH, ESH_W], F16, tag="esh")
    nc.sync.dma_start(out=esh, in_=t["eshift"].ap().rearrange("h p w -> p h w"))

    bias = {}
    for nm in ("b_tp", "b_q", "b_k", "b_tout", "b_vp", "b_i2tq",
               "b_t2ikv_v", "b_vout"):
        bt = cb.tile([P, KT], F32, tag=nm)
        nc.sync.dma_start(out=bt, in_=t[nm].ap())
        bias[nm] = bt
    b_i2tkv = cb.tile([P, KT], F32, tag="b_i2tkv")
    nc.sync.dma_start(out=b_i2tkv, in_=t["b_i2tkv"].ap())
    bias["b_i2tkv"] = b_i2tkv

    NSC = 12
    scbc = cb.tile([P, NSC], F32, tag="scbc")
    srcap = t["scal"].ap()
    nc.sync.dma_start(
        out=scbc,
        in_=bass.AP(tensor=srcap.tensor, offset=srcap.offset,
                    ap=[[0, P], [1, NSC]]))
    (SC_TP, SC_QKV, SC_I2TKV, SC_TOUT, SC_VP, SC_I2TQ, SC_T2IKV, SC_VOUT,
     AL_I2T, AL_T2I) = range(10)

    def sc(i):
        return scbc[:, i:i + 1]

    def wtile(dram, kk, mm, dt, tag, mw=P, coff=0):
        pool = wpa if tag == "wA" else wp
        wt = pool.tile([P, mw], dt, tag=tag)
        nc.sync.dma_start(
            out=wt,
            in_=dram.ap()[kk * P:(kk + 1) * P, coff + mm * mw:coff + (mm + 1) * mw])
        return wt

    def lin_splitA(dram, hi, lo, out_sb, scale_ap, bias_t, n):
        """out^T[:,m,:] = scale*(W^T.T @ (hi+lo)) + bias, ternary W in f32r."""
        for m in range(KT):
            pt = psA.tile([P, S], F32, tag="A")
            for k in range(KT):
                wt = wtile(dram, k, m, F32R, "wA")
                nc.tensor.matmul(pt[:, :n], lhsT=wt, rhs=hi[:, k, :],
                                 start=(k == 0), stop=False)
                nc.tensor.matmul(pt[:, :n], lhsT=wt, rhs=lo[:, k, :],
                                 start=False, stop=(k == KT - 1))
            nc.scalar.activation(out=out_sb[:, m, :], in_=pt[:, :n],
                                 func=ACTF.Identity, bias=bias_t[:, m:m + 1],
                                 scale=scale_ap)

    def split_hi_lo(src_sb, hi, lo):
        for m in range(KT):
            nc.scalar.activation(out=hi[:, m, :], in_=src_sb[:, m, :],
                                 func=ACTF.Copy)
            nc.vector.tensor_tensor(out=lo[:, m, :], in0=src_sb[:, m, :],
                                    in1=hi[:, m, :].bitcast(F32), op=OP.subtract)

    def ln_T(src, dst, n):
        """LayerNorm over the feature (partition) dim of src [128,KT,n]."""
        stat0 = psL.tile([1, S], F32, tag="L0")
        stat1 = psL.tile([1, S], F32, tag="L1")
        sq = ep.tile([P, S], F32, tag="ln_sq")
        for k in range(KT):
            nc.tensor.matmul(stat0[:, :n], lhsT=ones_col, rhs=src[:, k, :],
                             start=(k == 0), stop=(k == KT - 1))
        for k in range(KT):
            nc.scalar.activation(out=sq[:, :n], in_=src[:, k, :], func=ACTF.Square)
            nc.tensor.matmul(stat1[:, :n], lhsT=ones_col, rhs=sq[:, :n],
                             start=(k == 0), stop=(k == KT - 1))
        row0 = sm.tile([1, S], F32, tag="ln_row0")
        row1 = sm.tile([1, S], F32, tag="ln_row1")
        nc.vector.tensor_scalar(out=row0[:, :n], in0=stat0[:, :n],
                                scalar1=1.0 / H, scalar2=None, op0=OP.mult)
        nc.vector.tensor_scalar(out=row1[:, :n], in0=stat1[:, :n],
                                scalar1=1.0 / H, scalar2=None, op0=OP.mult)
        var = sm.tile([1, S], F32, tag="ln_var")
        nc.vector.tensor_tensor(out=var[:, :n], in0=row0[:, :n],
                                in1=row0[:, :n], op=OP.mult)
        nc.vector.tensor_tensor(out=var[:, :n], in0=row1[:, :n],
                                in1=var[:, :n], op=OP.subtract)
        sg = sm.tile([1, S], F32, tag="ln_sg")
        nc.scalar.activation(out=sg[:, :n], in_=var[:, :n], func=ACTF.Sqrt,
                             bias=eps_col[0:1, :])
        rs = sm.tile([1, S], F32, tag="ln_rs")
        nc.vector.reciprocal(out=rs[:, :n], in_=sg[:, :n])
        tmp = sm.tile([1, S], F32, tag="ln_tmp")
        nc.vector.tensor_tensor(out=tmp[:, :n], in0=sg[:, :n], in1=rs[:, :n],
                                op=OP.mult)
        nc.vector.tensor_scalar(out=tmp[:, :n], in0=tmp[:, :n], scalar1=-1.0,
                                scalar2=2.0, op0=OP.mult, op1=OP.add)
        nc.vector.tensor_tensor(out=rs[:, :n], in0=rs[:, :n], in1=tmp[:, :n],
                                op=OP.mult)
        mub = psL.tile([P, S], F32, tag="L0")
        rsb = psL.tile([P, S], F32, tag="L1")
        nc.tensor.matmul(mub[:, :n], lhsT=ones_row, rhs=row0[:, :n],
                         start=True, stop=True)
        nc.tensor.matmul(rsb[:, :n], lhsT=ones_row, rhs=rs[0:1, :n],
                         start=True, stop=True)
        for k in range(KT):
            nc.vector.tensor_tensor(out=dst[:, k, :], in0=src[:, k, :],
                                    in1=mub[:, :n], op=OP.subtract)
            nc.vector.tensor_tensor(out=dst[:, k, :], in0=dst[:, k, :],
                                    in1=rsb[:, :n], op=OP.mult)

    # ---------------- vision path (both batches, tiny) ----------------
    vis_nat = vp_.tile([BL, H], F32, tag="vis_nat")
    nc.sync.dma_start(out=vis_nat, in_=t["vision"].ap())
    visT = vp_.tile([P, KT, BL], F32, tag="visT")
    for k in range(KT):
        pt = psV.tile([P, NH], F32, tag="V")
        nc.tensor.transpose(pt[:, :BL], vis_nat[:, k * P:(k + 1) * P], ident_f[:BL, :BL])
        nc.vector.tensor_copy(visT[:, k, :], pt[:, :BL])

    def vis_lin(wname, bname, src, scale_i):
        dst = vp_.tile([P, KT, BL], F32, tag=f"vl_{wname}")
        for m in range(KT):
            pt = psV.tile([P, NH], F32, tag="V")
            for k in range(KT):
                wt = wtile(t[wname], k, m, F32, "wV")
                nc.tensor.matmul(pt[:, :BL], lhsT=wt, rhs=src[:, k, :],
                                 start=(k == 0), stop=(k == KT - 1))
            nc.scalar.activation(out=dst[:, m, :], in_=pt[:, :BL],
                                 func=ACTF.Identity, bias=bias[bname][:, m:m + 1],
                                 scale=sc(scale_i))
        return dst

    vpT = vis_lin("w_vp", "b_vp", visT, SC_VP)          # v_exp^T
    vlnT = vp_.tile([P, KT, BL], F32, tag="vlnT")
    ln_T(vpT, vlnT, BL)
    i2tqT = vis_lin("w_i2tq", "b_i2tq", vlnT, SC_I2TQ)
    t2ivT = vis_lin("w_t2ikv_v", "b_t2ikv_v", vpT, SC_T2IKV)
    a2T = vp_.tile([P, KT, BL], F32, tag="a2T")
    for k in range(KT):
        nc.vector.tensor_scalar(out=a2T[:, k, :], in0=t2ivT[:, k, :],
                                scalar1=sc(AL_T2I), scalar2=None, op0=OP.mult)
    vcrossT = vp_.tile([P, KT, BL], F32, tag="vcrossT")

    textr = t["text"].ap().rearrange("b (qt p) h -> b qt p h", p=P)

    # ---------------- text path (per batch) ----------------
    for b in range(BL):
        textT = big.tile([P, KT, S], F32, tag="bigA")
        for qt in range(QT):
            nat = ep.tile([P, H], F32, tag="t_nat")
            nc.sync.dma_start(out=nat, in_=textr[b, qt])
            for k in range(KT):
                pt = psT.tile([P, S], F32, tag="T")
                nc.tensor.transpose(pt[:, :P], nat[:, k * P:(k + 1) * P], ident_f)
                nc.vector.tensor_copy(textT[:, k, qt * P:(qt + 1) * P],
                                      pt[:, :P])
        x_hi = big.tile([P, KT, S], F32R, tag="bigB")
        x_lo = big.tile([P, KT, S], F32R, tag="bigC")
        split_hi_lo(textT, x_hi, x_lo)
        tpT = big.tile([P, KT, S], F32, tag="tpT")
        lin_splitA(t["w_tp"], x_hi, x_lo, tpT, sc(SC_TP), bias["b_tp"], S)
        tnT = big.tile([P, KT, S], F32, tag="bigA")
        ln_T(tpT, tnT, S)
        tn_hi = big.tile([P, KT, S], F32R, tag="bigB")
        tn_lo = big.tile([P, KT, S], F32R, tag="bigC")
        split_hi_lo(tnT, tn_hi, tn_lo)
        qT = big.tile([P, KT, S], F32, tag="qT")
        kTT = big.tile([P, KT, S], F32, tag="kT")
        lin_splitA(t["w_q"], tn_hi, tn_lo, qT, sc(SC_QKV), bias["b_q"], S)
        lin_splitA(t["w_k"], tn_hi, tn_lo, kTT, sc(SC_QKV), bias["b_k"], S)
        # v natural via FORM B (v bias is all-zero; asserted host-side)
        vN = big.tile([P, QT, H], F32, tag="vN")
        for qt in range(QT):
            for half in range(2):
                pt = psA.tile([P, S], F32, tag="A")
                for k in range(KT):
                    wt = wtile(t["w_v"], k, half, F32R, "wB", mw=384)
                    nc.tensor.matmul(pt[:, :384],
                                     lhsT=tn_hi[:, k, qt * P:(qt + 1) * P],
                                     rhs=wt, start=(k == 0), stop=False)
                    nc.tensor.matmul(pt[:, :384],
                                     lhsT=tn_lo[:, k, qt * P:(qt + 1) * P],
                                     rhs=wt, start=False, stop=(k == KT - 1))
                nc.scalar.activation(
                    out=vN[:, qt, half * 384:(half + 1) * 384],
                    in_=pt[:, :384], func=ACTF.Identity, scale=sc(SC_QKV))

        # ------------- self-attention -------------
        tsT = big.tile([P, KT, S], F32, tag="tsT")
        for h in range(NH):
            kt_i, pof = h // 2, (h % 2) * HD
            qh = qT[pof:pof + HD, kt_i, :]
            kh = kTT[pof:pof + HD, kt_i, :]
            EnT = enp.tile([P, QT, S], F32, tag="EnT")
            for qt in range(QT):
                spt = psA.tile([P, S], F32, tag="A")
                nc.tensor.matmul(spt, lhsT=qh[:, qt * P:(qt + 1) * P], rhs=kh,
                                 start=True, stop=True)
                sb_s = ep.tile([P, S], F32, tag="sb_s")
                nc.vector.tensor_scalar(out=sb_s, in0=spt, scalar1=ATT_SCALE,
                                        scalar2=None, op0=OP.mult)
                nc.vector.tensor_tensor(
                    out=sb_s, in0=sb_s,
                    in1=esh[:, h, 384 - P * qt: 384 - P * qt + S], op=OP.add)
                mx = sm.tile([P, 1], F32, tag="mx")
                nc.vector.reduce_max(mx, sb_s, axis=AX.X)
                nmx = sm.tile([P, 1], F32, tag="nmx")
                nc.vector.tensor_scalar(out=nmx, in0=mx, scalar1=-1.0,
                                        scalar2=None, op0=OP.mult)
                Et = ep.tile([P, S], F32, tag="Et")
                sums = sm.tile([P, 1], F32, tag="sums")
                nc.scalar.activation(out=Et, in_=sb_s, func=ACTF.Exp, bias=nmx,
                                     scale=1.0, accum_out=sums)
                rec = sm.tile([P, 1], F32, tag="rec")
                nc.vector.reciprocal(out=rec, in_=sums)
                t1 = sm.tile([P, 1], F32, tag="t1")
                nc.vector.tensor_tensor(out=t1, in0=sums, in1=rec, op=OP.mult)
                nc.vector.tensor_scalar(out=t1, in0=t1, scalar1=-1.0,
                                        scalar2=2.0, op0=OP.mult, op1=OP.add)
                nc.vector.tensor_tensor(out=rec, in0=rec, in1=t1, op=OP.mult)
                En = ep.tile([P, S], F32, tag="En")
                nc.vector.tensor_scalar(out=En, in0=Et, scalar1=rec,
                                        scalar2=None, op0=OP.mult)
                for c in range(QT):
                    ptt = psT.tile([P, S], F32, tag="T")
                    nc.tensor.transpose(ptt[:, :P], En[:, c * P:(c + 1) * P],
                                        ident_f)
                    nc.vector.tensor_copy(EnT[:, c, qt * P:(qt + 1) * P],
                                          ptt[:, :P])
            apt = psT.tile([P, S], F32, tag="T")
            for c in range(QT):
                nc.tensor.matmul(apt[:HD, :], lhsT=vN[:, c, h * HD:(h + 1) * HD],
                                 rhs=EnT[:, c, :], start=(c == 0),
                                 stop=(c == QT - 1))
            nc.vector.tensor_tensor(out=tsT[pof:pof + HD, kt_i, :],
                                    in0=apt[:HD, :],
                                    in1=tpT[pof:pof + HD, kt_i, :], op=OP.add)

        # ------------- i2t cross attention + outputs -------------
        tsr = big.tile([P, KT, S], F32R, tag="bigA")
        for k in range(KT):
            nc.scalar.activation(out=tsr[:, k, :], in_=tsT[:, k, :],
                                 func=ACTF.Copy)
        i2tkT = big.tile([P, KT, S], F32, tag="bigB")
        for m in range(KT):
            pt = psA.tile([P, S], F32, tag="A")
            for k in range(KT):
                wt = wtile(t["w_i2tkv"], k, m, F32R, "wA")
                nc.tensor.matmul(pt, lhsT=wt, rhs=tsr[:, k, :],
                                 start=(k == 0), stop=(k == KT - 1))
            nc.scalar.activation(out=i2tkT[:, m, :], in_=pt, func=ACTF.Identity,
                                 bias=bias["b_i2tkv"][:, m:m + 1],
                                 scale=sc(SC_I2TKV))
        i2tvN = big.tile([P, QT, H], F32, tag="bigC")
        for qt in range(QT):
            for half in range(2):
                pt = psA.tile([P, S], F32, tag="A")
                for k in range(KT):
                    wt = wtile(t["w_i2tkv"], k, half, F32R, "wB", mw=384,
                               coff=H)
                    nc.tensor.matmul(pt[:, :384],
                                     lhsT=tsr[:, k, qt * P:(qt + 1) * P],
                                     rhs=wt, start=(k == 0),
                                     stop=(k == KT - 1))
                nc.scalar.activation(
                    out=i2tvN[:, qt, half * 384:(half + 1) * 384],
                    in_=pt[:, :384], func=ACTF.Identity, scale=sc(SC_I2TKV))
        isb = sm.tile([NH, S], F32, tag="isb")
        for h in range(NH):
            kt_i, pof = h // 2, (h % 2) * HD
            ispt = psT.tile([1, S], F32, tag="T")
            nc.tensor.matmul(ispt,
                             lhsT=i2tqT[pof:pof + HD, kt_i, b:b + 1],
                             rhs=i2tkT[pof:pof + HD, kt_i, :],
                             start=True, stop=True)
            isr = sm.tile([1, S], F32, tag="isr")
            nc.vector.tensor_scalar(out=isr, in0=ispt, scalar1=ATT_SCALE,
                                    scalar2=None, op0=OP.mult)
            nc.sync.dma_start(out=isb[h:h + 1, :], in_=isr)
        imx = sm.tile([NH, 1], F32, tag="imx")
        nc.vector.reduce_max(imx, isb, axis=AX.X)
        nc.vector.tensor_scalar(out=imx, in0=imx, scalar1=-1.0, scalar2=None,
                                op0=OP.mult)
        iE = sm.tile([NH, S], F32, tag="iE")
        isums = sm.tile([NH, 1], F32, tag="isums")
        nc.scalar.activation(out=iE, in_=isb, func=ACTF.Exp, bias=imx,
                             scale=1.0, accum_out=isums)
        irec = sm.tile([NH, 1], F32, tag="irec")
        nc.vector.reciprocal(out=irec, in_=isums)
        nc.vector.tensor_scalar(out=iE, in0=iE, scalar1=irec, scalar2=None,
                                op0=OP.mult)
        iaT = sm.tile([P, QT, NH], F32, tag="iaT")
        for c in range(QT):
            ptt = psV.tile([P, NH], F32, tag="V")
            nc.tensor.transpose(ptt[:, :NH], iE[:, c * P:(c + 1) * P], ident_f[:NH, :NH])
            nc.vector.tensor_copy(iaT[:, c, :], ptt[:, :NH])
        for m in range(KT):
            pt = psV.tile([P, NH], F32, tag="V")
            for c in range(QT):
                nc.tensor.matmul(pt, lhsT=i2tvN[:, c, m * P:(m + 1) * P],
                                 rhs=iaT[:, c, :], start=(c == 0),
                                 stop=(c == QT - 1))
            for half in range(2):
                h = 2 * m + half
                nc.vector.tensor_scalar(
                    out=vcrossT[half * HD:(half + 1) * HD, m, b:b + 1],
                    in0=pt[half * HD:(half + 1) * HD, h:h + 1],
                    scalar1=scbc[half * HD:(half + 1) * HD, AL_I2T:AL_I2T + 1],
                    scalar2=None, op0=OP.mult)
                nc.vector.tensor_tensor(
                    out=vcrossT[half * HD:(half + 1) * HD, m, b:b + 1],
                    in0=vcrossT[half * HD:(half + 1) * HD, m, b:b + 1],
                    in1=vpT[half * HD:(half + 1) * HD, m, b:b + 1], op=OP.add)

        # text_cross = ts + alpha_t2i * t2i_v (broadcast over tokens), in place
        for k in range(KT):
            nc.vector.tensor_scalar(out=tsT[:, k, :], in0=tsT[:, k, :],
                                    scalar1=a2T[:, k, b:b + 1], scalar2=None,
                                    op0=OP.add)
        tc_hi = big.tile([P, KT, S], F32R, tag="bigA")
        tc_lo = big.tile([P, KT, S], F32R, tag="bigC")
        split_hi_lo(tsT, tc_hi, tc_lo)
        ftT = big.tile([P, KT, S], F32, tag="bigB")
        lin_splitA(t["w_tout"], tc_hi, tc_lo, ftT, sc(SC_TOUT),
                   bias["b_tout"], S)
        nc.sync.dma_start(
            out=t["out_ft"].ap().rearrange("b (kt p) s -> b p kt s", p=P)[b],
            in_=ftT)

    fvT = vis_lin("w_vout", "b_vout", vcrossT, SC_VOUT)
    nc.sync.dma_start(
        out=t["out_fv"].ap().rearrange("(kt p) b -> p kt b", p=P), in_=fvT)
    ctx.close()


def _build():
    nc = bacc.Bacc(None, target_bir_lowering=False)

    def din(name, shape, dt=F32):
        return nc.declare_dram_parameter(name, list(shape), dt, isOutput=False)

    t = {}
    t["text"] = din("text", [BL, S, H])
    t["vision"] = din("vision", [BL, H])
    t["w_tp"] = din("w_tp", [H, H], F32R)
    t["w_q"] = din("w_q", [H, H], F32R)
    t["w_k"] = din("w_k", [H, H], F32R)
    t["w_v"] = din("w_v", [H, H], F32R)
    t["w_i2tkv"] = din("w_i2tkv", [H, 2 * H], F32R)
    t["w_tout"] = din("w_tout", [H, H], F32R)
    t["w_vp"] = din("w_vp", [H, H])
    t["w_i2tq"] = din("w_i2tq", [H, H])
    t["w_t2ikv_v"] = din("w_t2ikv_v", [H, H])
    t["w_vout"] = din("w_vout", [H, H])
    t["eshift"] = din("eshift", [NH, P, ESH_W], F16)
    for nm in ("b_tp", "b_q", "b_k", "b_tout", "b_vp", "b_i2tq", "b_t2ikv_v",
               "b_vout", "b_i2tkv"):
        t[nm] = din(nm, [P, KT])
    t["scal"] = din("scal", [12, 1])
    t["out_ft"] = nc.declare_dram_parameter("out_ft", [BL, H, S], F32,
                                            isOutput=True)
    t["out_fv"] = nc.declare_dram_parameter("out_fv", [H, BL], F32,
                                            isOutput=True)
    with tile.TileContext(nc) as tc:
        _emit(nc, tc, t)
    nc.finalize()
    return nc


def kernel(**inputs):
    f32 = lambda x: np.ascontiguousarray(np.asarray(x, np.float32))
    for n in ("v", "t", "i2t", "t2i"):
        assert np.all(np.asarray(inputs[f"ln_{n}_g"]) == 1.0)
        assert np.all(np.asarray(inputs[f"ln_{n}_b"]) == 0.0)
    assert np.all(np.asarray(inputs["text_mask"]))
    assert np.all(np.asarray(inputs["tqkv_b"])[2 * H:] == 0)
    assert np.all(np.asarray(inputs["i2tkv_b"])[H:] == 0)

    nc = _CACHE.get("nc")
    if nc is None:
        nc = _build()
        _CACHE["nc"] = nc

    q_tp, s_tp = _quant(inputs["tp_w"])
    q_qkv, s_qkv = _quant(inputs["tqkv_w"])
    q_i2tkv, s_i2tkv = _quant(inputs["i2tkv_w"])
    q_tout, s_tout = _quant(inputs["tout_w"])
    q_vp, s_vp = _quant(inputs["vp_w"])
    q_i2tq, s_i2tq = _quant(inputs["i2tq_w"])
    q_t2ikv, s_t2ikv = _quant(inputs["t2ikv_w"])
    q_vout, s_vout = _quant(inputs["vout_w"])

    emb = f32(inputs["rel_embed"])
    jj = np.arange(ESH_W)[None, :]
    pp = np.arange(P)[:, None]
    idx = np.clip(pp - jj + 384, -MAX_REL, MAX_REL) + MAX_REL
    esh = np.ascontiguousarray(
        emb[idx][None].transpose(3, 1, 2, 0)[..., 0].astype(np.float16))
    # esh[h, p, j] = emb[idx[p, j], h]
    esh = np.ascontiguousarray(emb[idx, :].transpose(2, 0, 1).astype(np.float16))

    qkv_b = f32(inputs["tqkv_b"])
    scal = np.array([s_tp, s_qkv, s_i2tkv, s_tout, s_vp, s_i2tq, s_t2ikv,
                     s_vout, float(np.asarray(inputs["alpha_i2t"]).ravel()[0]),
                     float(np.asarray(inputs["alpha_t2i"]).ravel()[0]),
                     0.0, 0.0], np.float32).reshape(12, 1)

    base = {
        "w_tp": q_tp,
        "w_q": np.ascontiguousarray(q_qkv[:, :H]),
        "w_k": np.ascontiguousarray(q_qkv[:, H:2 * H]),
        "w_v": np.ascontiguousarray(q_qkv[:, 2 * H:]),
        "w_i2tkv": q_i2tkv, "w_tout": q_tout, "w_vp": q_vp,
        "w_i2tq": q_i2tq,
        "w_t2ikv_v": np.ascontiguousarray(q_t2ikv[:, H:]),
        "w_vout": q_vout, "eshift": esh, "scal": scal,
        "b_tp": _col_tiles(inputs["tp_b"]),
        "b_q": _col_tiles(qkv_b[:H]),
        "b_k": _col_tiles(qkv_b[H:2 * H]),
        "b_i2tkv": _col_tiles(f32(inputs["i2tkv_b"])[:H]),
        "b_tout": _col_tiles(inputs["tout_b"]),
        "b_vp": _col_tiles(inputs["vp_b"]),
        "b_i2tq": _col_tiles(inputs["i2tq_b"]),
        "b_t2ikv_v": _col_tiles(f32(inputs["t2ikv_b"])[H:]),
        "b_vout": _col_tiles(inputs["vout_b"]),
    }
    text = f32(inputs["text_features"])
    visf = f32(inputs["vision_features"])
    in_maps = []
    for c in range(NCORES):
        m = dict(base)
        m["text"] = np.ascontiguousarray(text[c * BL:(c + 1) * BL])
        m["vision"] = np.ascontiguousarray(visf[c * BL:(c + 1) * BL])
        in_maps.append(m)
    _CACHE["in_maps"] = in_maps

    res = run_bass_kernel_spmd(nc, in_maps, core_ids=list(range(NCORES)))
    fv = np.empty((B, H), np.float32)
    ft = np.empty((B, S, H), np.float32)
    for c in range(NCORES):
        r = res.results[c]
        fv[c * BL:(c + 1) * BL] = r["out_fv"].T
        ft[c * BL:(c + 1) * BL] = r["out_ft"].transpose(0, 2, 1)
    return fv, ft
